# revision 25
# baseline (speedup 1.0000x reference)
"""Trainium2 Bass kernel for the Ewald energy nn.Module.

Math restructuring (validated to ~1e-7 rel err against the jax reference in
fp32; ~4e-5 with the fp16 fast path):
  E = E_real + E_recip + E_self with the charge contraction folded in:
    E_real  = 0.5*COEF * sum_s sum_ij q_i q_j (erf(r/(sqrt2*gam_ij)) - erf(r/(sqrt2*eta)))/r
              over the symmetric half of the 6x6 grid of 128-atom tile pairs
              (off-diagonal pairs weighted 2x).  d^2 comes from a Gram-matrix
              matmul on the TensorEngine; atoms are pre-sorted along x so
              (tile-pair, x-shift-group) units whose x-gap exceeds the
              real-space cutoff are culled exactly (the reference masks
              r>cutoff).  The (i==i, s=0) self-pair is removed by setting
              invgamma[i,i] = 1/(sqrt2*eta) so the erf difference cancels.
    E_recip = 0.5*COEF*(4pi/V) * sum_k w_k * (Sc_k^2 + Ss_k^2), structure
              factors via PE matmuls; only the ~1646 nonzero-weight k of the
              17^3 grid are computed.  theta is range-reduced via fractional
              coordinates and the (y + 1.5*2^23) - 1.5*2^23 round trick so
              Sin stays inside its [-pi, pi] table range.
    E_self  = diagonal term, O(N).

Sharding: surviving real-space units are distributed round-robin over the 8
cores (UPC slots each, dummies zero-weighted); active k split 256 per core;
every core returns a scalar partial and the host sums the 8.

ScalarEngine activations are phase-ordered (Abs_reciprocal_sqrt -> Erf ->
Sin) via emission order plus explicit scheduler edges through two tiny ACT
barrier copies, so each ACT table set loads exactly once.  The erf pipeline
runs in fp16 to unlock the DVE 2x perf mode.
"""
import math
import os
import sys
import numpy as np

_TRN_REPO = "/opt/trn_rl_repo"
if _TRN_REPO not in sys.path and os.path.isdir(_TRN_REPO):
    sys.path.insert(0, _TRN_REPO)

COEF = 14.399645478425668
N = 768
NT = 6            # 128-atom tiles
P = 128
NSHIFT_REAL = 1   # -> 27 shifts, 3 x-groups of 9
NSG = 3
SG = 9
UCOL = P * SG     # 1152 columns per unit
CH = 384          # matmul chunk
CHS = 512         # psum chunk stride (bank aligned)
NCH = 3
NCORES = 8
NSHIFT_RECIP = 8  # -> 17^3 = 4913 k-vectors; ~1646 carry weight
KPC = 256         # active k per core
_MAX_UNITS = 48   # provable upper bound on surviving units

_nc_cache = {}
_runner_cache = {}


class _CachedRunner:
    """One-time jit of the 8-core shard_map bass_exec dispatch.

    Mirrors concourse.bass2jax.run_bass_via_pjrt (the axon redirect target
    of bass_utils.run_bass_kernel_spmd) but builds the jitted callable once
    and reuses it: the stock path re-creates jax.jit(shard_map(closure))
    every call, which re-traces + re-lowers (~150-200 ms/call of pure host
    overhead on top of the ~85 ms axon dispatch round-trip).
    """

    def __init__(self, nc, n_cores):
        import jax
        import numpy as np
        from jax.sharding import Mesh, PartitionSpec
        from jax.experimental.shard_map import shard_map
        from concourse import mybir
        from concourse.bass2jax import (_bass_exec_p, partition_id_tensor,
                                        install_neuronx_cc_hook)

        install_neuronx_cc_hook()
        self.n_cores = n_cores
        pname = nc.partition_id_tensor.name if nc.partition_id_tensor else None
        in_names, out_names, out_avals, zero_outs = [], [], [], []
        for alloc in nc.m.functions[0].allocations:
            if not isinstance(alloc, mybir.MemoryLocationSet):
                continue
            name = alloc.memorylocations[0].name
            if alloc.kind == "ExternalInput":
                if name != pname:
                    in_names.append(name)
            elif alloc.kind == "ExternalOutput":
                shape = tuple(alloc.tensor_shape)
                dtype = mybir.dt.np(alloc.dtype)
                out_names.append(name)
                out_avals.append(jax.core.ShapedArray(shape, dtype))
                zero_outs.append(np.zeros(shape, dtype))
        self.in_names, self.out_names = in_names, out_names
        self.zero_outs = zero_outs
        n_params, n_outs = len(in_names), len(out_avals)
        self.n_params = n_params
        in_names_all = in_names + out_names + ([pname] if pname else [])
        donate = tuple(range(n_params, n_params + n_outs))

        def _body(*args):
            operands = list(args)
            if pname is not None:
                operands.append(partition_id_tensor())
            outs = _bass_exec_p.bind(
                *operands, out_avals=tuple(out_avals),
                in_names=tuple(in_names_all), out_names=tuple(out_names),
                lowering_input_output_aliases=(),
                sim_require_finite=True, sim_require_nnan=True, nc=nc)
            return tuple(outs)

        devices = jax.devices()[:n_cores]
        assert len(devices) == n_cores
        mesh = Mesh(np.asarray(devices), ("core",))
        in_specs = (PartitionSpec("core"),) * (n_params + n_outs)
        out_specs = (PartitionSpec("core"),) * len(out_names)
        self.fn = jax.jit(
            shard_map(_body, mesh=mesh, in_specs=in_specs,
                      out_specs=out_specs, check_rep=False),
            donate_argnums=donate, keep_unused=True)
        from jax.sharding import NamedSharding
        self.sharding = NamedSharding(mesh, PartitionSpec("core"))
        self._jax = jax
        self._small_cache = {}   # name -> (np copy, device array)

    def __call__(self, in_maps):
        """in_maps: list of n_cores dicts name->np.ndarray.  Returns list of
        per-core dicts name->np.ndarray, as run_bass_kernel_spmd does."""
        per_core = [[np.asarray(m[name]) for name in self.in_names]
                    for m in in_maps]
        concat_in = [
            np.concatenate([per_core[c][i] for c in range(self.n_cores)], axis=0)
            for i in range(self.n_params)]
        zeros = [np.zeros((self.n_cores * z.shape[0], *z.shape[1:]), z.dtype)
                 for z in self.zero_outs]
        outs = self.fn(*concat_in, *zeros)
        outs = [np.asarray(o) for o in outs]
        return [
            {name: outs[i].reshape(self.n_cores, *self.zero_outs[i].shape)[c]
             for i, name in enumerate(self.out_names)}
            for c in range(self.n_cores)]

    def call_packed(self, packed):
        """packed: dict name -> already axis-0-concatenated [8*rows, cols]
        array.  Returns the list of per-core output dicts.  One retry on a
        transient device error (wedged NRT exec unit recovers on re-run).

        Inputs ship as host arrays every call: pre-staging them on device
        was measured slower on this transport (the per-call host buffer
        store of the large input is what the axon fast path keys on)."""
        concat_in = [packed[name] for name in self.in_names]
        for attempt in range(2):
            try:
                zeros = [np.zeros((self.n_cores * z.shape[0], *z.shape[1:]),
                                  z.dtype) for z in self.zero_outs]
                outs = [np.asarray(o) for o in self.fn(*concat_in, *zeros)]
                break
            except Exception:
                if attempt == 1:
                    raise
                import time as _time
                _time.sleep(2.0)
        return [
            {name: outs[i].reshape(self.n_cores, *self.zero_outs[i].shape)[c]
             for i, name in enumerate(self.out_names)}
            for c in range(self.n_cores)]


def get_runner(c1, upc, version=4):
    key = (round(float(c1), 12), int(upc), version)
    if key not in _runner_cache:
        if version == 4:
            nc = build_program2(c1, upc, packed_io="single")
        elif version == 3:
            nc = build_program2(c1, upc, packed_io=True)
        elif version == 2:
            nc = build_program2(c1, upc, packed_io=False)
        else:
            nc = build_program(c1, upc)
        _runner_cache[key] = _CachedRunner(nc, NCORES)
    return _runner_cache[key]


def _pack_spec(upc):
    """(name, rows, cols) of each logical input inside the packed tensors."""
    s32 = [("lhsu", 5, upc * P), ("aj3", 3, upc * P), ("shc", 3, upc * SG),
           ("s2i2", 2, upc * P), ("s2j2", 2, upc * P), ("vc", P, upc),
           ("qj2", upc, P), ("mT", 4, 2 * KPC), ("fT", 4, N),
           ("w2", 2, KPC), ("sd", P, NT), ("qmat", P, NT)]
    s16 = [("q2c", P, upc), ("qmat16", P, NT)]
    return s32, s16


def _pack_spec4(upc):
    """Like _pack_spec but every f16 piece has an even column count so its
    rows bitcast cleanly to f32 words inside the single packed tensor."""
    s32, _ = _pack_spec(upc)
    s16 = [("q2c", P, upc + (upc % 2)), ("qmat16", P, NT)]
    return s32, s16


def _tile_pairs():
    return [(ti, tj) for ti in range(NT) for tj in range(ti, NT)]


def _select_units(a, shifts, cutoff):
    """Cull (tilepair, shift-group) units whose x-gap exceeds the real-space
    cutoff.  Requires atoms sorted by x; sorted blocks guarantee at most 48
    survivors."""
    x = a[:, 0]
    lo = [x[t * P:(t + 1) * P].min() for t in range(NT)]
    hi = [x[t * P:(t + 1) * P].max() for t in range(NT)]
    units = []
    for (ti, tj) in _tile_pairs():
        for sg in range(NSG):
            keep = False
            for s in range(SG):
                sx = float(shifts[sg * SG + s, 0])
                d_lo = lo[tj] + sx - hi[ti]
                d_hi = hi[tj] + sx - lo[ti]
                if not (d_lo > cutoff or d_hi < -cutoff):
                    keep = True
                    break
            if keep:
                units.append((ti, tj, sg))
    assert len(units) <= _MAX_UNITS, f"{len(units)} units > {_MAX_UNITS}"
    return units


def build_program(c1, upc):
    """Build + compile the per-core Bass program (same on all cores).

    c1 = 1/(sqrt(2)*eta) is baked in as the erf scale constant; upc is the
    number of real-space unit slots per core.
    """
    key = ("nc", round(float(c1), 12), int(upc))
    if key in _nc_cache:
        return _nc_cache[key]

    import concourse.bacc as bacc
    import concourse.tile as tile
    from concourse import mybir
    from concourse.tile import add_dep_helper

    AF = mybir.ActivationFunctionType
    OP = mybir.AluOpType
    f32 = mybir.dt.float32
    f16 = mybir.dt.float16

    nc = bacc.Bacc("TRN2", target_bir_lowering=False, debug=False)

    dt_in = {}

    def din(name, shape, dtype=f32):
        dt_in[name] = nc.dram_tensor(name, shape, dtype,
                                     kind="ExternalInput").ap()
        return dt_in[name]

    din("lhsu", [5, upc * P])
    din("rhsu", [5, upc * UCOL])
    din("invgu", [P, upc * P], f16)
    din("q2c", [P, upc], f16)
    din("qjr", [upc, UCOL])
    din("mT", [4, 2 * KPC])
    din("fT", [4, N])
    din("w2", [2, KPC])
    din("sd", [P, NT])
    din("qmat", [P, NT])
    din("qmat16", [P, NT], f16)
    out_d = nc.dram_tensor("out", [1, 1], f32, kind="ExternalOutput").ap()

    with tile.TileContext(nc) as tc:
        with tc.tile_pool(name="consts", bufs=1) as cp, \
             tc.tile_pool(name="rall", bufs=1) as rp, \
             tc.tile_pool(name="trig", bufs=6) as tp_, \
             tc.tile_pool(name="work", bufs=2) as wp, \
             tc.tile_pool(name="stage", bufs=2) as stp, \
             tc.tile_pool(name="scps", bufs=1, space="PSUM") as scp, \
             tc.tile_pool(name="bigps", bufs=2, space="PSUM") as bigp, \
             tc.tile_pool(name="thps", bufs=1, space="PSUM") as thp:

            # ---- load inputs to SBUF ----
            sb = {}
            for name, ap in dt_in.items():
                t = cp.tile(list(ap.shape), ap.dtype, name=f"sb_{name}")
                nc.sync.dma_start(t[:], ap[:])
                sb[name] = t

            beps = cp.tile([P, 1], f32)
            nc.vector.memset(beps[:], 1e-8)
            ones_t = cp.tile([P, 1], f32)
            nc.vector.memset(ones_t[:], 1.0)

            rall = rp.tile([P, upc * UCOL], f16)   # 1/r for all units
            arows = cp.tile([upc, UCOL], f32)      # per-unit reduced rows
            scs = cp.tile([2, KPC], f32)           # [Sc; Ss] (squared later)
            scs_st = cp.tile([1, 2 * KPC], f32)    # partition-0 staging

            # ============ recip theta + range reduction (early) ============
            scargs = []
            RC = 12582912.0  # 1.5 * 2**23: (y + RC) - RC == round-to-nearest
            for t in range(NT):
                th = thp.tile([P, 2 * KPC], f32, tag="th")
                nc.tensor.matmul(th[:, 0:KPC], sb["fT"][:, t * P:(t + 1) * P],
                                 sb["mT"][:, 0:KPC], start=True, stop=True)
                nc.tensor.matmul(th[:, KPC:], sb["fT"][:, t * P:(t + 1) * P],
                                 sb["mT"][:, KPC:], start=True, stop=True)
                scarg = tp_.tile([P, 2 * KPC], f32, tag="scarg")
                rnd = stp.tile([P, 2 * KPC], f32, tag="rnd")
                nc.vector.tensor_scalar(rnd[:], th[:], RC, RC,
                                        OP.add, OP.subtract)
                nc.vector.tensor_tensor(scarg[:], th[:], rnd[:], OP.subtract)
                scargs.append(scarg)

            # ============ real part, phase 1: d2 + 1/r ============
            # one 3-bank psum tile per unit; 3 Gram matmuls at bank-aligned
            # offsets, drained by a single strided Abs_reciprocal_sqrt
            sqrt_instrs = []
            for u in range(upc):
                d2 = bigp.tile([P, NCH * CHS], f32, tag="big")
                for ch in range(NCH):
                    nc.tensor.matmul(d2[:, ch * CHS:ch * CHS + CH],
                                     sb["lhsu"][:, u * P:(u + 1) * P],
                                     sb["rhsu"][:, u * UCOL + ch * CH:
                                                u * UCOL + (ch + 1) * CH],
                                     start=True, stop=True)
                si = nc.scalar.activation(
                    rall[:, u * UCOL:(u + 1) * UCOL]
                        .rearrange("p (c f) -> p c f", c=NCH),
                    d2[:].rearrange("p (c f) -> p c f", c=NCH)[:, :, 0:CH],
                    AF.Abs_reciprocal_sqrt, bias=beps[:], scale=1.0)
                if sqrt_instrs:
                    add_dep_helper(si.ins, sqrt_instrs[-1].ins, sync=False,
                                   reason="sqrt unit order")
                sqrt_instrs.append(si)

            # ---- ACT barrier 1: abs_rsqrt -> erf ----
            bar1t = cp.tile([1, 1], f32)
            b1 = nc.scalar.copy(bar1t[:], ones_t[0:1, :])
            for s in sqrt_instrs:
                add_dep_helper(b1.ins, s.ins, sync=False, reason="act sqrt->erf")

            # ============ real part, phase 2: erf pipeline (fp16) ============
            erf_instrs = []
            for u in range(upc):
                rinv_u = rall[:, u * UCOL:(u + 1) * UCOL]
                H = UCOL // 2
                r_u = wp.tile([P, UCOL], f16, tag="r")
                with nc.allow_low_precision(reason="fp16 erf pipeline"):
                    nc.vector.reciprocal(r_u[:, 0:H], rinv_u[:, 0:H])
                    nc.vector.reciprocal(r_u[:, H:], rinv_u[:, H:])
                erf1 = wp.tile([P, UCOL], f16, tag="erf1")
                for hs in (slice(0, H), slice(H, UCOL)):
                    e1 = nc.scalar.activation(erf1[:, hs], r_u[:, hs], AF.Erf,
                                              bias=0.0, scale=float(c1))
                    add_dep_helper(e1.ins, b1.ins, sync=False,
                                   reason="act sqrt->erf")
                    erf_instrs.append(e1)
                arg2 = wp.tile([P, UCOL], f16, tag="arg2")
                # columns are s-major (col = s*128 + j): broadcast invg over s
                # with the unit-stride j innermost so DVE 2x mode applies
                invg_b = sb["invgu"][:, u * P:(u + 1) * P].unsqueeze(1) \
                    .broadcast_to([P, SG, P])
                nc.vector.tensor_tensor(
                    arg2[:].rearrange("p (s j) -> p s j", s=SG),
                    r_u[:].rearrange("p (s j) -> p s j", s=SG),
                    invg_b, OP.mult)
                erf2 = wp.tile([P, UCOL], f16, tag="erf2")
                e2 = nc.scalar.activation(erf2[:], arg2[:], AF.Erf,
                                          bias=0.0, scale=1.0)
                add_dep_helper(e2.ins, b1.ins, sync=False, reason="act sqrt->erf")
                erf_instrs.append(e2)
                # diff and valr in place (erf2 <- erf2-erf1 on GPSIMD,
                # erf1 <- diff*rinv on DVE)
                nc.gpsimd.tensor_tensor(erf2[:, 0:H], erf2[:, 0:H],
                                        erf1[:, 0:H], OP.subtract)
                nc.vector.tensor_tensor(erf2[:, H:], erf2[:, H:],
                                        erf1[:, H:], OP.subtract)
                nc.vector.tensor_tensor(erf1[:], erf2[:], rinv_u, OP.mult)
                ast = stp.tile([1, UCOL], f32, tag="ast")
                red = bigp.tile([1, NCH * CHS], f32, tag="big")
                for ch in range(NCH):
                    nc.tensor.matmul(red[0:1, ch * CHS:ch * CHS + CH],
                                     sb["q2c"][:, u:u + 1],
                                     erf1[:, ch * CH:(ch + 1) * CH],
                                     start=True, stop=True)
                nc.vector.tensor_copy(
                    ast[:].rearrange("p (c f) -> p c f", c=NCH),
                    red[:].rearrange("p (c f) -> p c f", c=NCH)[:, :, 0:CH])
                nc.sync.dma_start(arows[u:u + 1, :], ast[:])

            racc_r = cp.tile([upc, 1], f32)
            trash_r = cp.tile([upc, UCOL], f32)
            nc.vector.tensor_tensor(trash_r[:], arows[:], sb["qjr"][:], OP.mult)
            nc.vector.tensor_reduce(racc_r[:], trash_r[:],
                                    axis=mybir.AxisListType.X, op=OP.add)

            # ---- ACT barrier 2: erf -> sin ----
            bar2t = cp.tile([1, 1], f32)
            b2 = nc.scalar.copy(bar2t[:], ones_t[0:1, :])
            for e in erf_instrs:
                add_dep_helper(b2.ins, e.ins, sync=False, reason="act erf->sin")

            # ============ reciprocal part ============
            # psum row: [Sc | Ss] accumulators in one bank
            scrow = scp.tile([1, 2 * KPC], f32, tag="sc")
            sncs = []
            for t in range(NT):
                snc = tp_.tile([P, 2 * KPC], f16, tag="snc")
                si = nc.scalar.activation(snc[:], scargs[t][:], AF.Sin,
                                          bias=0.0, scale=float(2 * math.pi))
                add_dep_helper(si.ins, b2.ins, sync=False, reason="act erf->sin")
                sncs.append(snc)
            for t in range(NT):
                nc.tensor.matmul(scrow[0:1, 0:KPC], sb["qmat16"][:, t:t + 1],
                                 sncs[t][:, KPC:],
                                 start=(t == 0), stop=(t == NT - 1))
            for t in range(NT):
                nc.tensor.matmul(scrow[0:1, KPC:], sb["qmat16"][:, t:t + 1],
                                 sncs[t][:, 0:KPC],
                                 start=(t == 0), stop=(t == NT - 1))
            nc.scalar.copy(scs_st[:], scrow[:])
            nc.sync.dma_start(scs[:], scs_st[:])

            sqk = cp.tile([2, KPC], f32)
            nc.vector.tensor_tensor(sqk[:], scs[:], scs[:], OP.mult)
            racc_k = cp.tile([2, 1], f32)
            trash_k = cp.tile([2, KPC], f32)
            nc.vector.tensor_tensor(trash_k[:], sqk[:], sb["w2"][:], OP.mult)
            nc.vector.tensor_reduce(racc_k[:], trash_k[:],
                                    axis=mybir.AxisListType.X, op=OP.add)

            # ============ self part ============
            q2t = cp.tile([P, NT], f32)
            nc.vector.tensor_tensor(q2t[:], sb["qmat"][:], sb["qmat"][:], OP.mult)
            sacc = cp.tile([P, 1], f32)
            trash_s = cp.tile([P, NT], f32)
            nc.vector.tensor_tensor(trash_s[:], q2t[:], sb["sd"][:], OP.mult)
            nc.vector.tensor_reduce(sacc[:], trash_s[:],
                                    axis=mybir.AxisListType.X, op=OP.add)

            # ============ combine ============
            nc.vector.tensor_tensor(sacc[0:upc, :], sacc[0:upc, :],
                                    racc_r[:], OP.add)
            nc.vector.tensor_tensor(sacc[0:2, :], sacc[0:2, :],
                                    racc_k[:], OP.add)
            fin = thp.tile([1, 1], f32, tag="th")
            nc.tensor.matmul(fin[:], sacc[:], ones_t[:], start=True, stop=True)
            outt = cp.tile([1, 1], f32)
            nc.vector.tensor_copy(outt[:], fin[:])
            nc.sync.dma_start(out_d[:], outt[:])

    nc.compile()
    _nc_cache[key] = nc
    return nc


def build_program2(c1, upc, packed_io=True):
    """v2 program: the big real-space operands (rhsu, invgu, and the charge
    row for the j-side reduction) are constructed ON DEVICE from ~70KB of
    gathered per-unit inputs, instead of being transferred (~300KB/core).
    The downstream compute (d2 Gram, erf pipeline, reciprocal, self) is
    identical to build_program.  With packed_io the 14 logical inputs travel
    as two flat dram tensors (one f32, one f16) to cut per-array dispatch
    overhead on the PJRT path."""
    key = ("nc2", round(float(c1), 12), int(upc), bool(packed_io))
    if key in _nc_cache:
        return _nc_cache[key]

    import concourse.bacc as bacc
    import concourse.tile as tile
    from concourse import mybir
    from concourse.tile import add_dep_helper

    AF = mybir.ActivationFunctionType
    OP = mybir.AluOpType
    f32 = mybir.dt.float32
    f16 = mybir.dt.float16

    nc = bacc.Bacc("TRN2", target_bir_lowering=False, debug=False)

    dt_in = {}

    def din(name, shape, dtype=f32):
        dt_in[name] = nc.dram_tensor(name, shape, dtype,
                                     kind="ExternalInput").ap()
        return dt_in[name]

    if packed_io == "single":
        s32, s16 = _pack_spec4(upc)
        W32 = sum(r * c for _, r, c in s32)
        W16 = sum(r * c for _, r, c in s16)
        din("pk", [1, W32 + W16 // 2])
    else:
        s32, s16 = _pack_spec(upc)
        if packed_io:
            W32 = sum(r * c for _, r, c in s32)
            W16 = sum(r * c for _, r, c in s16)
            din("pk32", [1, W32])
            din("pk16", [1, W16], f16)
        else:
            for nm, r, c in s32:
                din(nm, [r, c])
            for nm, r, c in s16:
                din(nm, [r, c], f16)
    out_d = nc.dram_tensor("out", [1, 1], f32, kind="ExternalOutput").ap()

    with tile.TileContext(nc) as tc:
        with tc.tile_pool(name="consts", bufs=1) as cp, \
             tc.tile_pool(name="rall", bufs=1) as rp, \
             tc.tile_pool(name="trig", bufs=6) as tp_, \
             tc.tile_pool(name="work", bufs=2) as wp, \
             tc.tile_pool(name="stage", bufs=2) as stp, \
             tc.tile_pool(name="scps", bufs=1, space="PSUM") as scp, \
             tc.tile_pool(name="bigps", bufs=1, space="PSUM") as bigp, \
             tc.tile_pool(name="invps", bufs=2, space="PSUM") as ivp, \
             tc.tile_pool(name="thps", bufs=1, space="PSUM") as thp:

            # ---- load inputs to SBUF ----
            sb = {}
            if packed_io == "single":
                off = 0
                for nm, r, c in s32:
                    t = cp.tile([r, c], f32, name=f"sb_{nm}")
                    src = dt_in["pk"][0:1, off:off + r * c] \
                        .rearrange("p (a b) -> (p a) b", a=r, b=c)
                    nc.sync.dma_start(t[:], src)
                    sb[nm] = t
                    off += r * c
                for nm, r, c in s16:
                    t = cp.tile([r, c], f16, name=f"sb_{nm}")
                    w = (r * c) // 2
                    src = dt_in["pk"][0:1, off:off + w] \
                        .rearrange("p (a b) -> (p a) b", a=r, b=c // 2)
                    nc.sync.dma_start(t[:].bitcast(f32), src)
                    sb[nm] = t
                    off += w
            elif packed_io:
                for (pkname, spec, dt) in (("pk32", s32, f32),
                                           ("pk16", s16, f16)):
                    off = 0
                    for nm, r, c in spec:
                        t = cp.tile([r, c], dt, name=f"sb_{nm}")
                        src = dt_in[pkname][0:1, off:off + r * c] \
                            .rearrange("p (a b) -> (p a) b", a=r, b=c)
                        nc.sync.dma_start(t[:], src)
                        sb[nm] = t
                        off += r * c
            else:
                for name, ap in dt_in.items():
                    t = cp.tile(list(ap.shape), ap.dtype, name=f"sb_{name}")
                    nc.sync.dma_start(t[:], ap[:])
                    sb[name] = t

            beps = cp.tile([P, 1], f32)
            nc.vector.memset(beps[:], 1e-8)
            ones_t = cp.tile([P, 1], f32)
            nc.vector.memset(ones_t[:], 1.0)

            rall = rp.tile([P, upc * UCOL], f16)   # 1/r for all units
            arows = cp.tile([upc, UCOL], f32)      # per-unit reduced rows
            scs = cp.tile([2, KPC], f32)           # [Sc; Ss] (squared later)
            scs_st = cp.tile([1, 2 * KPC], f32)    # partition-0 staging

            # ============ device-side operand construction ============
            # rhsu rows 0-2: b = a_j + s (scaled -2 in place after squaring);
            # row 3: |b|^2, row 4: ones (lhsu rows are [x,y,z,1,a2] to match).
            # Compute-engine writes must start at partition 0, so rows 3-4
            # are staged in a partition-0 tile and DMA'd into place.
            rhsu = cp.tile([5, upc * UCOL], f32)
            sqt = cp.tile([3, upc * UCOL], f32)
            onesb2 = cp.tile([2, upc * UCOL], f32)
            ones3 = cp.tile([3, 1], f32)
            nc.vector.memset(ones3[:], 1.0)
            nc.vector.memset(onesb2[:], 1.0)
            for u in range(upc):
                for s in range(SG):
                    nc.vector.tensor_scalar_add(
                        rhsu[0:3, u * UCOL + s * P: u * UCOL + (s + 1) * P],
                        sb["aj3"][:, u * P:(u + 1) * P],
                        sb["shc"][:, u * SG + s: u * SG + s + 1])
            nc.vector.tensor_tensor(sqt[:], rhsu[0:3, :], rhsu[0:3, :],
                                    OP.mult)
            for u in range(upc):
                bq = bigp.tile([1, NCH * CHS], f32, tag="big")
                for ch in range(NCH):
                    nc.tensor.matmul(bq[0:1, ch * CHS:ch * CHS + CH],
                                     ones3[:], sqt[:, u * UCOL + ch * CH:
                                                   u * UCOL + (ch + 1) * CH],
                                     start=True, stop=True)
                nc.vector.tensor_copy(
                    onesb2[0:1, u * UCOL:(u + 1) * UCOL]
                        .rearrange("p (c f) -> p c f", c=NCH),
                    bq[:].rearrange("p (c f) -> p c f", c=NCH)[:, :, 0:CH])
            nc.vector.tensor_scalar_mul(rhsu[0:3, :], rhsu[0:3, :], -2.0)
            nc.sync.dma_start(rhsu[3:5, :], onesb2[:])

            # identity matrix for the invg diagonal fix
            eye = cp.tile([P, P], f32)
            nc.gpsimd.memset(eye[:], 1.0)
            nc.gpsimd.affine_select(
                out=eye[:], in_=eye[:], compare_op=OP.is_equal, fill=0.0,
                base=0, pattern=[[-1, P]], channel_multiplier=1)

            # ============ recip theta + range reduction (early) ============
            scargs = []
            RC = 12582912.0  # 1.5 * 2**23: (y + RC) - RC == round-to-nearest
            for t in range(NT):
                th = thp.tile([P, 2 * KPC], f32, tag="th")
                nc.tensor.matmul(th[:, 0:KPC], sb["fT"][:, t * P:(t + 1) * P],
                                 sb["mT"][:, 0:KPC], start=True, stop=True)
                nc.tensor.matmul(th[:, KPC:], sb["fT"][:, t * P:(t + 1) * P],
                                 sb["mT"][:, KPC:], start=True, stop=True)
                scarg = tp_.tile([P, 2 * KPC], f32, tag="scarg")
                rnd = stp.tile([P, 2 * KPC], f32, tag="rnd")
                nc.vector.tensor_scalar(rnd[:], th[:], RC, RC,
                                        OP.add, OP.subtract)
                nc.vector.tensor_tensor(scarg[:], th[:], rnd[:], OP.subtract)
                scargs.append(scarg)

            # ============ real part, phase 1: d2 + 1/r ============
            sqrt_instrs = []
            for u in range(upc):
                d2 = bigp.tile([P, NCH * CHS], f32, tag="big")
                for ch in range(NCH):
                    nc.tensor.matmul(d2[:, ch * CHS:ch * CHS + CH],
                                     sb["lhsu"][:, u * P:(u + 1) * P],
                                     rhsu[:, u * UCOL + ch * CH:
                                          u * UCOL + (ch + 1) * CH],
                                     start=True, stop=True)
                si = nc.scalar.activation(
                    rall[:, u * UCOL:(u + 1) * UCOL]
                        .rearrange("p (c f) -> p c f", c=NCH),
                    d2[:].rearrange("p (c f) -> p c f", c=NCH)[:, :, 0:CH],
                    AF.Abs_reciprocal_sqrt, bias=beps[:], scale=1.0)
                if sqrt_instrs:
                    add_dep_helper(si.ins, sqrt_instrs[-1].ins, sync=False,
                                   reason="sqrt unit order")
                sqrt_instrs.append(si)

            # ---- invg on device (same ACT table as the d2 rsqrt) ----
            invgu = cp.tile([P, upc * P], f16)
            for u in range(upc):
                psA = ivp.tile([P, P], f32, tag="ps")
                nc.tensor.matmul(psA[:], sb["s2i2"][:, u * P:(u + 1) * P],
                                 sb["s2j2"][:, u * P:(u + 1) * P],
                                 start=True, stop=True)
                ec = wp.tile([P, P], f32, tag="ec")
                nc.vector.tensor_scalar_mul(ec[:], eye[:], sb["vc"][:, u:u + 1])
                nc.vector.tensor_tensor(psA[:], psA[:], ec[:], OP.add)
                gi = nc.scalar.activation(invgu[:, u * P:(u + 1) * P], psA[:],
                                          AF.Abs_reciprocal_sqrt,
                                          bias=0.0, scale=2.0)
                sqrt_instrs.append(gi)

            # ---- ACT barrier 1: abs_rsqrt -> erf ----
            bar1t = cp.tile([1, 1], f32)
            b1 = nc.scalar.copy(bar1t[:], ones_t[0:1, :])
            for s in sqrt_instrs:
                add_dep_helper(b1.ins, s.ins, sync=False, reason="act sqrt->erf")

            # ============ real part, phase 2: erf pipeline (fp16) ============
            erf_instrs = []
            for u in range(upc):
                rinv_u = rall[:, u * UCOL:(u + 1) * UCOL]
                H = UCOL // 2
                r_u = wp.tile([P, UCOL], f16, tag="r")
                with nc.allow_low_precision(reason="fp16 erf pipeline"):
                    nc.vector.reciprocal(r_u[:, 0:H], rinv_u[:, 0:H])
                    nc.vector.reciprocal(r_u[:, H:], rinv_u[:, H:])
                erf1 = wp.tile([P, UCOL], f16, tag="erf1")
                for hs in (slice(0, H), slice(H, UCOL)):
                    e1 = nc.scalar.activation(erf1[:, hs], r_u[:, hs], AF.Erf,
                                              bias=0.0, scale=float(c1))
                    add_dep_helper(e1.ins, b1.ins, sync=False,
                                   reason="act sqrt->erf")
                    erf_instrs.append(e1)
                arg2 = wp.tile([P, UCOL], f16, tag="arg2")
                invg_b = invgu[:, u * P:(u + 1) * P].unsqueeze(1) \
                    .broadcast_to([P, SG, P])
                nc.vector.tensor_tensor(
                    arg2[:].rearrange("p (s j) -> p s j", s=SG),
                    r_u[:].rearrange("p (s j) -> p s j", s=SG),
                    invg_b, OP.mult)
                erf2 = wp.tile([P, UCOL], f16, tag="erf2")
                e2 = nc.scalar.activation(erf2[:], arg2[:], AF.Erf,
                                          bias=0.0, scale=1.0)
                add_dep_helper(e2.ins, b1.ins, sync=False, reason="act sqrt->erf")
                erf_instrs.append(e2)
                nc.gpsimd.tensor_tensor(erf2[:, 0:H], erf2[:, 0:H],
                                        erf1[:, 0:H], OP.subtract)
                nc.vector.tensor_tensor(erf2[:, H:], erf2[:, H:],
                                        erf1[:, H:], OP.subtract)
                nc.vector.tensor_tensor(erf1[:], erf2[:], rinv_u, OP.mult)
                ast = stp.tile([1, UCOL], f32, tag="ast")
                red = bigp.tile([1, NCH * CHS], f32, tag="big")
                for ch in range(NCH):
                    nc.tensor.matmul(red[0:1, ch * CHS:ch * CHS + CH],
                                     sb["q2c"][:, u:u + 1],
                                     erf1[:, ch * CH:(ch + 1) * CH],
                                     start=True, stop=True)
                nc.vector.tensor_copy(
                    ast[:].rearrange("p (c f) -> p c f", c=NCH),
                    red[:].rearrange("p (c f) -> p c f", c=NCH)[:, :, 0:CH])
                nc.sync.dma_start(arows[u:u + 1, :], ast[:])

            # j-side charge contraction: reduce over shifts first, then q_j
            ar_s = cp.tile([upc, P], f32)
            nc.vector.tensor_reduce(
                ar_s[:], arows[:].rearrange("u (s j) -> u j s", s=SG),
                axis=mybir.AxisListType.X, op=OP.add)
            racc_r = cp.tile([upc, 1], f32)
            trash_r = cp.tile([upc, P], f32)
            nc.vector.tensor_tensor(trash_r[:], ar_s[:], sb["qj2"][:], OP.mult)
            nc.vector.tensor_reduce(racc_r[:], trash_r[:],
                                    axis=mybir.AxisListType.X, op=OP.add)

            # ---- ACT barrier 2: erf -> sin ----
            bar2t = cp.tile([1, 1], f32)
            b2 = nc.scalar.copy(bar2t[:], ones_t[0:1, :])
            for e in erf_instrs:
                add_dep_helper(b2.ins, e.ins, sync=False, reason="act erf->sin")

            # ============ reciprocal part ============
            scrow = scp.tile([1, 2 * KPC], f32, tag="sc")
            sncs = []
            for t in range(NT):
                snc = tp_.tile([P, 2 * KPC], f16, tag="snc")
                si = nc.scalar.activation(snc[:], scargs[t][:], AF.Sin,
                                          bias=0.0, scale=float(2 * math.pi))
                add_dep_helper(si.ins, b2.ins, sync=False, reason="act erf->sin")
                sncs.append(snc)
            for t in range(NT):
                nc.tensor.matmul(scrow[0:1, 0:KPC], sb["qmat16"][:, t:t + 1],
                                 sncs[t][:, KPC:],
                                 start=(t == 0), stop=(t == NT - 1))
            for t in range(NT):
                nc.tensor.matmul(scrow[0:1, KPC:], sb["qmat16"][:, t:t + 1],
                                 sncs[t][:, 0:KPC],
                                 start=(t == 0), stop=(t == NT - 1))
            nc.scalar.copy(scs_st[:], scrow[:])
            nc.sync.dma_start(scs[:], scs_st[:])

            sqk = cp.tile([2, KPC], f32)
            nc.vector.tensor_tensor(sqk[:], scs[:], scs[:], OP.mult)
            racc_k = cp.tile([2, 1], f32)
            trash_k = cp.tile([2, KPC], f32)
            nc.vector.tensor_tensor(trash_k[:], sqk[:], sb["w2"][:], OP.mult)
            nc.vector.tensor_reduce(racc_k[:], trash_k[:],
                                    axis=mybir.AxisListType.X, op=OP.add)

            # ============ self part ============
            q2t = cp.tile([P, NT], f32)
            nc.vector.tensor_tensor(q2t[:], sb["qmat"][:], sb["qmat"][:], OP.mult)
            sacc = cp.tile([P, 1], f32)
            trash_s = cp.tile([P, NT], f32)
            nc.vector.tensor_tensor(trash_s[:], q2t[:], sb["sd"][:], OP.mult)
            nc.vector.tensor_reduce(sacc[:], trash_s[:],
                                    axis=mybir.AxisListType.X, op=OP.add)

            # ============ combine ============
            nc.vector.tensor_tensor(sacc[0:upc, :], sacc[0:upc, :],
                                    racc_r[:], OP.add)
            nc.vector.tensor_tensor(sacc[0:2, :], sacc[0:2, :],
                                    racc_k[:], OP.add)
            fin = thp.tile([1, 1], f32, tag="th")
            nc.tensor.matmul(fin[:], sacc[:], ones_t[:], start=True, stop=True)
            outt = cp.tile([1, 1], f32)
            nc.vector.tensor_copy(outt[:], fin[:])
            nc.sync.dma_start(out_d[:], outt[:])

    nc.compile()
    _nc_cache[key] = nc
    return nc


_shift_grid_cache = {}


def _shift_grid(n):
    # pure function of n (and n is asserted constant), so cache the lattice
    if n not in _shift_grid_cache:
        r = np.arange(-n, n + 1, dtype=np.float64)
        g = np.stack(np.meshgrid(r, r, r, indexing="ij"), axis=-1)
        _shift_grid_cache[n] = g.reshape(-1, 3)
    return _shift_grid_cache[n]


def prep_in_maps(pos, cell, charges, sigma_table, species_idx):
    """Host-side shard prep: returns (in_maps list of 8 dicts, c1, upc)."""
    pos = np.asarray(pos, np.float32)
    cell = np.asarray(cell, np.float32)
    if cell.ndim == 3:
        cell = cell[0]
    q = np.asarray(charges, np.float32).reshape(-1)
    sigma_table = np.asarray(sigma_table, np.float32)
    species_idx = np.asarray(species_idx).astype(np.int64)
    sigmas = sigma_table[species_idx]

    vol = abs(np.linalg.det(cell.astype(np.float64)))
    eta = (vol ** 2 / N) ** (1.0 / 6.0) / math.sqrt(2.0 * math.pi)
    cutoff_recip = math.sqrt(-2.0 * math.log(1e-8)) / eta
    cutoff_real = math.sqrt(-2.0 * math.log(1e-8)) * eta
    c1 = 1.0 / (math.sqrt(2.0) * eta)

    # sort atoms along x so the 128-atom tiles become x-slabs (enables exact
    # culling of far tile-pair/shift units)
    perm = np.argsort(pos[:, 0], kind="stable")
    pos = pos[perm]
    q = q[perm]
    sigmas = sigmas[perm]

    center = 0.5 * cell.astype(np.float64).sum(axis=0)
    a = (pos.astype(np.float64) - center).astype(np.float32)
    a2 = (a * a).sum(1).astype(np.float32)
    shifts = (_shift_grid(NSHIFT_REAL) @ cell.astype(np.float64)).astype(np.float32)

    sig2 = sigmas.astype(np.float32) ** 2
    invg = (1.0 / np.sqrt(2.0 * (sig2[:, None] + sig2[None, :]))).astype(np.float32)
    np.fill_diagonal(invg, np.float32(c1))

    units = _select_units(a, shifts, cutoff_real)
    upc = max(1, (len(units) + NCORES - 1) // NCORES)
    units = units + [None] * (NCORES * upc - len(units))

    # reciprocal k-grid: keep only k with nonzero weight (exact culling)
    gk = _shift_grid(NSHIFT_RECIP)                     # (4913, 3) float64
    recip = 2.0 * math.pi * np.linalg.inv(cell.astype(np.float64)).T
    ks_all = gk @ recip
    klen_all = np.linalg.norm(ks_all, axis=-1)
    kmask = (klen_all > 1e-8) & (klen_all < cutoff_recip)
    kidx = np.nonzero(kmask)[0]
    KTOT = NCORES * KPC
    assert len(kidx) <= KTOT, f"{len(kidx)} active k > {KTOT} slots"
    gk_pad = np.zeros((KTOT, 3), np.float64)
    gk_pad[: len(kidx)] = gk[kidx]
    wk = np.zeros(KTOT, np.float64)
    wk[: len(kidx)] = (np.exp(-0.5 * (eta * klen_all[kidx]) ** 2)
                       / klen_all[kidx] ** 2)
    wk = wk * (0.5 * COEF * 4.0 * math.pi / vol)
    frac = pos.astype(np.float64) @ np.linalg.inv(cell.astype(np.float64))
    fT_all = np.ones((4, N), np.float32)
    fT_all[0:3] = frac.T.astype(np.float32)            # row 3 stays 1.0

    # self part
    diag = (-math.sqrt(2.0 / math.pi) / eta
            + 1.0 / (math.sqrt(math.pi) * sigmas.astype(np.float64)))
    sd_all = (diag * 0.5 * COEF / NCORES).astype(np.float32)

    in_maps = []
    for c in range(NCORES):
        lhsu = np.zeros((5, upc * P), np.float32)
        rhsu = np.zeros((5, upc * UCOL), np.float32)
        invgu = np.zeros((P, upc * P), np.float32)
        q2c = np.zeros((P, upc), np.float32)
        qjr = np.zeros((upc, UCOL), np.float32)
        for k in range(upc):
            unit = units[c * upc + k]
            if unit is None:
                ti, tj, sg = 0, 0, 0
                wu = 0.0
            else:
                ti, tj, sg = unit
                wu = 1.0 if ti == tj else 2.0
            ai = a[ti * P:(ti + 1) * P]                # (128, 3)
            lhsu[0:3, k * P:(k + 1) * P] = ai.T
            lhsu[3, k * P:(k + 1) * P] = a2[ti * P:(ti + 1) * P]
            lhsu[4, k * P:(k + 1) * P] = 1.0
            aj = a[tj * P:(tj + 1) * P]                # (128, 3)
            # s-major columns: col = s*128 + j
            b = shifts[sg * SG:(sg + 1) * SG, None, :] + aj[None, :, :]
            b2v = (b * b).sum(-1)                      # (9, 128)
            cols = slice(k * UCOL, (k + 1) * UCOL)
            rhsu[0:3, cols] = (-2.0 * b).reshape(UCOL, 3).T
            rhsu[3, cols] = 1.0
            rhsu[4, cols] = b2v.reshape(UCOL)
            invgu[:, k * P:(k + 1) * P] = invg[ti * P:(ti + 1) * P,
                                               tj * P:(tj + 1) * P]
            q2c[:, k] = q[ti * P:(ti + 1) * P] * np.float32(0.5 * COEF * wu)
            if unit is not None:
                qjr[k] = np.tile(q[tj * P:(tj + 1) * P], SG)
        ksl = slice(c * KPC, (c + 1) * KPC)
        mTc = np.empty((4, 2 * KPC), np.float32)
        mTc[0:3, 0:KPC] = gk_pad[ksl].T.astype(np.float32)
        mTc[0:3, KPC:] = mTc[0:3, 0:KPC]
        mTc[3, 0:KPC] = 24.0      # sin columns: theta/2pi + 24
        mTc[3, KPC:] = 24.25      # cos columns: + quarter period
        w2c = np.broadcast_to(wk[ksl].astype(np.float32), (2, KPC)).copy()
        in_maps.append({
            "lhsu": lhsu, "rhsu": rhsu,
            "invgu": invgu.astype(np.float16),
            "q2c": q2c.astype(np.float16), "qjr": qjr,
            "mT": mTc, "fT": fT_all.copy(), "w2": w2c,
            "sd": sd_all.reshape(NT, P).T.copy(),
            "qmat": q.reshape(NT, P).T.copy(),
            "qmat16": q.reshape(NT, P).T.astype(np.float16),
        })
    return in_maps, c1, upc


def prep_packed(pos, cell, charges, sigma_table, species_idx):
    """Vectorized host-side prep.  Produces the same per-core inputs as
    prep_in_maps but directly as axis-0-concatenated arrays (the layout the
    sharded jit call consumes), with no per-unit python loops."""
    pos = np.asarray(pos, np.float32)
    cell = np.asarray(cell, np.float32)
    if cell.ndim == 3:
        cell = cell[0]
    q = np.asarray(charges, np.float32).reshape(-1)
    sigma_table = np.asarray(sigma_table, np.float32)
    species_idx = np.asarray(species_idx).astype(np.int64)
    sigmas = sigma_table[species_idx]

    vol = abs(np.linalg.det(cell.astype(np.float64)))
    eta = (vol ** 2 / N) ** (1.0 / 6.0) / math.sqrt(2.0 * math.pi)
    cutoff_recip = math.sqrt(-2.0 * math.log(1e-8)) / eta
    cutoff_real = math.sqrt(-2.0 * math.log(1e-8)) * eta
    c1 = 1.0 / (math.sqrt(2.0) * eta)

    perm = np.argsort(pos[:, 0], kind="stable")
    pos = pos[perm]
    q = q[perm]
    sigmas = sigmas[perm]

    center = 0.5 * cell.astype(np.float64).sum(axis=0)
    a = (pos.astype(np.float64) - center).astype(np.float32)
    a2 = (a * a).sum(1).astype(np.float32)
    shifts = (_shift_grid(NSHIFT_REAL) @ cell.astype(np.float64)).astype(np.float32)

    units = _select_units(a, shifts, cutoff_real)
    nu = len(units)
    upc = max(1, (nu + NCORES - 1) // NCORES)
    total = NCORES * upc
    TI = np.zeros(total, np.int64)
    TJ = np.zeros(total, np.int64)
    SGv = np.zeros(total, np.int64)
    WU = np.zeros(total, np.float32)
    for idx, (ti, tj, sg) in enumerate(units):
        TI[idx], TJ[idx], SGv[idx] = ti, tj, sg
        WU[idx] = 1.0 if ti == tj else 2.0
    act = np.zeros(total, np.float32)
    act[:nu] = 1.0

    A3 = a.reshape(NT, P, 3)
    A2r = a2.reshape(NT, P)
    Qr = q.reshape(NT, P)
    S3 = shifts.reshape(NSG, SG, 3)

    lhs = np.empty((total, 5, P), np.float32)
    lhs[:, 0:3, :] = A3[TI].transpose(0, 2, 1)
    lhs[:, 3, :] = A2r[TI]
    lhs[:, 4, :] = 1.0
    G_lhsu = np.ascontiguousarray(
        lhs.reshape(NCORES, upc, 5, P).transpose(0, 2, 1, 3)
    ).reshape(NCORES * 5, upc * P)

    AJ = A3[TJ]                              # (T, 128, 3)
    SHu = S3[SGv]                            # (T, 9, 3)
    rhs = np.empty((total, 5, SG, P), np.float32)
    rhs[:, 0:3] = -2.0 * (AJ.transpose(0, 2, 1)[:, :, None, :]
                          + SHu.transpose(0, 2, 1)[:, :, :, None])
    rhs[:, 3] = 1.0
    dots = np.matmul(SHu, AJ.transpose(0, 2, 1))      # (T, 9, 128) = s.a_j
    sh2 = (SHu * SHu).sum(-1)                         # (T, 9)
    rhs[:, 4] = A2r[TJ][:, None, :] + 2.0 * dots + sh2[:, :, None]
    G_rhsu = np.ascontiguousarray(
        rhs.reshape(NCORES, upc, 5, UCOL).transpose(0, 2, 1, 3)
    ).reshape(NCORES * 5, upc * UCOL)

    sig2 = (sigmas.astype(np.float32) ** 2).reshape(NT, P)
    argb = sig2[TI][:, :, None] + sig2[TJ][:, None, :]  # (T, 128, 128)
    invgb = 1.0 / np.sqrt(2.0 * argb)
    dset = np.nonzero(TI == TJ)[0]
    ar = np.arange(P)
    invgb[dset[:, None], ar[None, :], ar[None, :]] = np.float32(c1)
    G_invgu = np.ascontiguousarray(
        invgb.reshape(NCORES, upc, P, P).transpose(0, 2, 1, 3)
    ).reshape(NCORES * P, upc * P).astype(np.float16)

    qb = Qr[TI] * (np.float32(0.5 * COEF) * WU)[:, None]
    G_q2c = np.ascontiguousarray(
        qb.reshape(NCORES, upc, P).transpose(0, 2, 1)
    ).reshape(NCORES * P, upc).astype(np.float16)

    qjsrc = Qr[TJ] * act[:, None]                     # (T, 128)
    G_qjr = np.ascontiguousarray(
        np.broadcast_to(qjsrc[:, None, :], (total, SG, P))
    ).reshape(total, UCOL)

    # ---- reciprocal ----
    gk = _shift_grid(NSHIFT_RECIP)
    recip = 2.0 * math.pi * np.linalg.inv(cell.astype(np.float64)).T
    ks_all = gk @ recip
    klen_all = np.linalg.norm(ks_all, axis=-1)
    kmask = (klen_all > 1e-8) & (klen_all < cutoff_recip)
    kidx = np.nonzero(kmask)[0]
    KTOT = NCORES * KPC
    assert len(kidx) <= KTOT, f"{len(kidx)} active k > {KTOT} slots"
    gk_pad = np.zeros((KTOT, 3), np.float64)
    gk_pad[: len(kidx)] = gk[kidx]
    wk = np.zeros(KTOT, np.float64)
    wk[: len(kidx)] = (np.exp(-0.5 * (eta * klen_all[kidx]) ** 2)
                       / klen_all[kidx] ** 2)
    wk = wk * (0.5 * COEF * 4.0 * math.pi / vol)

    mT_all = np.empty((NCORES, 4, 2 * KPC), np.float32)
    gkT = gk_pad.reshape(NCORES, KPC, 3).transpose(0, 2, 1).astype(np.float32)
    mT_all[:, 0:3, 0:KPC] = gkT
    mT_all[:, 0:3, KPC:] = gkT
    mT_all[:, 3, 0:KPC] = 24.0      # sin columns: theta/2pi + 24
    mT_all[:, 3, KPC:] = 24.25     # cos columns: + quarter period
    G_mT = mT_all.reshape(NCORES * 4, 2 * KPC)

    frac = pos.astype(np.float64) @ np.linalg.inv(cell.astype(np.float64))
    fT = np.ones((4, N), np.float32)
    fT[0:3] = frac.T
    G_fT = np.ascontiguousarray(
        np.broadcast_to(fT[None], (NCORES, 4, N))).reshape(NCORES * 4, N)

    w32 = wk.astype(np.float32).reshape(NCORES, 1, KPC)
    G_w2 = np.ascontiguousarray(
        np.broadcast_to(w32, (NCORES, 2, KPC))).reshape(NCORES * 2, KPC)

    diag = (-math.sqrt(2.0 / math.pi) / eta
            + 1.0 / (math.sqrt(math.pi) * sigmas.astype(np.float64)))
    sdT = (diag * 0.5 * COEF / NCORES).astype(np.float32).reshape(NT, P).T
    G_sd = np.ascontiguousarray(
        np.broadcast_to(sdT[None], (NCORES, P, NT))).reshape(NCORES * P, NT)

    qT = np.ascontiguousarray(Qr.T)
    G_qmat = np.ascontiguousarray(
        np.broadcast_to(qT[None], (NCORES, P, NT))).reshape(NCORES * P, NT)
    G_qmat16 = G_qmat.astype(np.float16)

    packed = {"lhsu": G_lhsu, "rhsu": G_rhsu, "invgu": G_invgu,
              "q2c": G_q2c, "qjr": G_qjr, "mT": G_mT, "fT": G_fT,
              "w2": G_w2, "sd": G_sd, "qmat": G_qmat, "qmat16": G_qmat16}
    return packed, c1, upc


def prep_packed2(pos, cell, charges, sigma_table, species_idx):
    """Host prep for build_program2: small gathered per-unit inputs only
    (~70KB/core); the big real-space operands are built on device."""
    pos = np.asarray(pos, np.float32)
    cell = np.asarray(cell, np.float32)
    if cell.ndim == 3:
        cell = cell[0]
    q = np.asarray(charges, np.float32).reshape(-1)
    sigma_table = np.asarray(sigma_table, np.float32)
    species_idx = np.asarray(species_idx).astype(np.int64)
    sigmas = sigma_table[species_idx]

    vol = abs(np.linalg.det(cell.astype(np.float64)))
    eta = (vol ** 2 / N) ** (1.0 / 6.0) / math.sqrt(2.0 * math.pi)
    cutoff_recip = math.sqrt(-2.0 * math.log(1e-8)) / eta
    cutoff_real = math.sqrt(-2.0 * math.log(1e-8)) * eta
    c1 = 1.0 / (math.sqrt(2.0) * eta)

    perm = np.argsort(pos[:, 0], kind="stable")
    pos = pos[perm]
    q = q[perm]
    sigmas = sigmas[perm]

    center = 0.5 * cell.astype(np.float64).sum(axis=0)
    a = (pos.astype(np.float64) - center).astype(np.float32)
    a2 = (a * a).sum(1).astype(np.float32)
    shifts = (_shift_grid(NSHIFT_REAL) @ cell.astype(np.float64)).astype(np.float32)

    units = _select_units(a, shifts, cutoff_real)
    nu = len(units)
    upc = max(1, (nu + NCORES - 1) // NCORES)
    total = NCORES * upc
    TI = np.zeros(total, np.int64)
    TJ = np.zeros(total, np.int64)
    SGv = np.zeros(total, np.int64)
    WU = np.zeros(total, np.float32)
    for idx, (ti, tj, sg) in enumerate(units):
        TI[idx], TJ[idx], SGv[idx] = ti, tj, sg
        WU[idx] = 1.0 if ti == tj else 2.0
    act = np.zeros(total, np.float32)
    act[:nu] = 1.0

    A3 = a.reshape(NT, P, 3)
    A2r = a2.reshape(NT, P)
    Qr = q.reshape(NT, P)
    S3 = shifts.reshape(NSG, SG, 3)

    def percore(blocks, rows):
        # blocks: (total, rows, X) -> concat layout [NCORES*rows, upc*X]
        X = blocks.shape[2]
        return np.ascontiguousarray(
            blocks.reshape(NCORES, upc, rows, X).transpose(0, 2, 1, 3)
        ).reshape(NCORES * rows, upc * X)

    # rows [x, y, z, 1, a2]: row 3 pairs with the on-device |b|^2 row,
    # row 4 with the on-device ones row
    lhs = np.empty((total, 5, P), np.float32)
    lhs[:, 0:3, :] = A3[TI].transpose(0, 2, 1)
    lhs[:, 3, :] = 1.0
    lhs[:, 4, :] = A2r[TI]
    G_lhsu = percore(lhs, 5)

    G_aj3 = percore(A3[TJ].transpose(0, 2, 1), 3)
    G_shc = percore(S3[SGv].transpose(0, 2, 1), 3)

    sig2 = (sigmas.astype(np.float32) ** 2).reshape(NT, P)
    s2i2 = np.empty((total, 2, P), np.float32)
    s2i2[:, 0, :] = sig2[TI]
    s2i2[:, 1, :] = 1.0
    G_s2i2 = percore(s2i2, 2)
    s2j2 = np.empty((total, 2, P), np.float32)
    s2j2[:, 0, :] = 1.0
    s2j2[:, 1, :] = sig2[TJ]
    G_s2j2 = percore(s2j2, 2)

    # diag-unit correction: (eye * vc) added to sigma_i^2+sigma_j^2 makes the
    # post-rsqrt diagonal exactly c1 = 1/(sqrt2 * eta)
    vcb = np.where((TI == TJ)[:, None],
                   np.float32(eta * eta) - 2.0 * sig2[TI],
                   np.float32(0.0)).astype(np.float32)       # (T, 128)
    G_vc = np.ascontiguousarray(
        vcb.reshape(NCORES, upc, P).transpose(0, 2, 1)).reshape(NCORES * P, upc)

    G_qj2 = np.ascontiguousarray(Qr[TJ] * act[:, None])      # (T, 128)

    qb = Qr[TI] * (np.float32(0.5 * COEF) * WU)[:, None]
    G_q2c = np.ascontiguousarray(
        qb.reshape(NCORES, upc, P).transpose(0, 2, 1)
    ).reshape(NCORES * P, upc).astype(np.float16)

    # ---- reciprocal ----
    gk = _shift_grid(NSHIFT_RECIP)
    recip = 2.0 * math.pi * np.linalg.inv(cell.astype(np.float64)).T
    ks_all = gk @ recip
    klen_all = np.linalg.norm(ks_all, axis=-1)
    kmask = (klen_all > 1e-8) & (klen_all < cutoff_recip)
    kidx = np.nonzero(kmask)[0]
    KTOT = NCORES * KPC
    assert len(kidx) <= KTOT, f"{len(kidx)} active k > {KTOT} slots"
    gk_pad = np.zeros((KTOT, 3), np.float64)
    gk_pad[: len(kidx)] = gk[kidx]
    wk = np.zeros(KTOT, np.float64)
    wk[: len(kidx)] = (np.exp(-0.5 * (eta * klen_all[kidx]) ** 2)
                       / klen_all[kidx] ** 2)
    wk = wk * (0.5 * COEF * 4.0 * math.pi / vol)

    mT_all = np.empty((NCORES, 4, 2 * KPC), np.float32)
    gkT = gk_pad.reshape(NCORES, KPC, 3).transpose(0, 2, 1).astype(np.float32)
    mT_all[:, 0:3, 0:KPC] = gkT
    mT_all[:, 0:3, KPC:] = gkT
    mT_all[:, 3, 0:KPC] = 24.0
    mT_all[:, 3, KPC:] = 24.25
    G_mT = mT_all.reshape(NCORES * 4, 2 * KPC)

    frac = pos.astype(np.float64) @ np.linalg.inv(cell.astype(np.float64))
    fT = np.ones((4, N), np.float32)
    fT[0:3] = frac.T
    G_fT = np.ascontiguousarray(
        np.broadcast_to(fT[None], (NCORES, 4, N))).reshape(NCORES * 4, N)

    w32 = wk.astype(np.float32).reshape(NCORES, 1, KPC)
    G_w2 = np.ascontiguousarray(
        np.broadcast_to(w32, (NCORES, 2, KPC))).reshape(NCORES * 2, KPC)

    diag = (-math.sqrt(2.0 / math.pi) / eta
            + 1.0 / (math.sqrt(math.pi) * sigmas.astype(np.float64)))
    sdT = (diag * 0.5 * COEF / NCORES).astype(np.float32).reshape(NT, P).T
    G_sd = np.ascontiguousarray(
        np.broadcast_to(sdT[None], (NCORES, P, NT))).reshape(NCORES * P, NT)

    qT = np.ascontiguousarray(Qr.T)
    G_qmat = np.ascontiguousarray(
        np.broadcast_to(qT[None], (NCORES, P, NT))).reshape(NCORES * P, NT)
    G_qmat16 = G_qmat.astype(np.float16)

    packed = {"lhsu": G_lhsu, "aj3": G_aj3, "shc": G_shc, "s2i2": G_s2i2,
              "s2j2": G_s2j2, "vc": G_vc, "qj2": G_qj2, "q2c": G_q2c,
              "mT": G_mT, "fT": G_fT, "w2": G_w2, "sd": G_sd,
              "qmat": G_qmat, "qmat16": G_qmat16}
    return packed, c1, upc


def prep_packed3(pos, cell, charges, sigma_table, species_idx):
    """prep_packed2 outputs flattened into the two packed input tensors."""
    packed, c1, upc = prep_packed2(pos, cell, charges, sigma_table,
                                   species_idx)
    s32, s16 = _pack_spec(upc)
    out = {}
    for pkname, spec, npdt in (("pk32", s32, np.float32),
                               ("pk16", s16, np.float16)):
        W = sum(r * c for _, r, c in spec)
        pk = np.empty((NCORES, W), npdt)
        off = 0
        for nm, r, c in spec:
            pk[:, off:off + r * c] = packed[nm].reshape(NCORES, r * c)
            off += r * c
        out[pkname] = pk
    return out, c1, upc


def prep_packed4(pos, cell, charges, sigma_table, species_idx):
    """Everything in ONE packed f32 tensor; f16 pieces ride as f32 words."""
    packed, c1, upc = prep_packed2(pos, cell, charges, sigma_table,
                                   species_idx)
    s32, s16 = _pack_spec4(upc)
    W32 = sum(r * c for _, r, c in s32)
    W16w = sum(r * c for _, r, c in s16) // 2
    pk = np.empty((NCORES, W32 + W16w), np.float32)
    off = 0
    for nm, r, c in s32:
        pk[:, off:off + r * c] = packed[nm].reshape(NCORES, r * c)
        off += r * c
    for nm, r, c in s16:
        arr = packed[nm]                     # [NCORES*r, c_orig] f16
        if arr.shape[1] != c:
            pad = np.zeros((arr.shape[0], c), np.float16)
            pad[:, :arr.shape[1]] = arr
            arr = pad
        w = (r * c) // 2
        pk[:, off:off + w] = np.ascontiguousarray(
            arr.reshape(NCORES, r * c)).view(np.float32)
        off += w
    return {"pk": pk}, c1, upc


def kernel(pos, cell, charges, sigma_table, species_idx,
           nshift_real, nshift_recip):
    assert int(nshift_real) == NSHIFT_REAL and int(nshift_recip) == NSHIFT_RECIP, \
        "kernel compiled for nshift_real=1, nshift_recip=8"
    pos = np.asarray(pos)
    assert pos.shape == (N, 3)

    packed, c1, upc = prep_packed4(pos, cell, charges, sigma_table,
                                   species_idx)
    runner = get_runner(c1, upc, version=4)
    results = runner.call_packed(packed)
    e = np.float64(0.0)
    for i in range(NCORES):
        e += np.float64(results[i]["out"][0, 0])
    return np.array([[e]], dtype=np.float32)

